# revision 15
# baseline (speedup 1.0000x reference)
"""AttentionAggregation kernel for 8 TRN2 NeuronCores (v3).

Math: out[b] = mean_n softmax(Q K^T)[n,:] @ V  with Q/K/V = x @ W^T + b.
Fold: out[b,d] = sum_m w[b,m] V[b,m,d],  w[b,m] = (1/N) sum_n exp(S[n,m])/R[n],
R[n] = sum_m exp(S[n,m]).  attn@V collapses to rank-1 matmuls (r^T @ E) plus a
single weighted reduction against V.  Softmax max-subtraction skipped (|S|<~25).

Sharding: core c -> batch b=c//2, softmax-row half h=c%2 (2048 rows each).
Host permutes x[b].T columns so each core's own row-half comes first (the m
axis is consistently permuted for K/V; softmax and the final sum are
permutation-invariant).  Host sums the two per-core partials and adds bv
(exact: each core's sum_m w[m] = 0.5).

v3 pipeline notes (trace-driven):
- ACT is the pacing engine: one 2048-wide exp = (2048+352)/1.2 = 2.0us, two
  per 128-row tile + 2 READ_ACC = 4.57us/tile floor.
- PSUM = two persistent [128,2048] tiles L (banks 0-3) / R (banks 4-7).
  Tile-framework deps are PSUM-tile-granular, so everything touching L
  serializes: exp_L(i) -> w(i-1,0) -> cast -> S(i+1,0) -> exp_L(i+1).
  v3 shortens that chain: the w psum is CAST (not read-modify-added) to a
  bf16 slot on DVE, and the accumulation into wacc happens on the otherwise
  idle GPSIMD engine, off the chain.  rmat copies also live on GPSIMD.
- V projection is fused into the epilogue (V chunks computed from xt right
  before the final multiply), so the main loop/prologue carry no V work.
- x arrives as 8 contiguous [128,512] pieces (1KB DMA elements instead of
  512B strided rows) issued from sync+scalar+gpsimd queues in parallel.
- PE HAM warm-up dummies + early exp-table load run during the DMA wait.

HW notes (inherited):
- everything PE-facing is bf16 (fp32 matmuls lower to HI/LO pairs).
- no DVE/ACT writes to PSUM banks that matmuls later accumulate into.
- tensor_tensor_reduce faults on HW; keep mult and reduce separate.
"""

import sys

sys.path.insert(0, "/opt/trn_rl_repo")

import ml_dtypes
import numpy as np

import concourse.bass as bass
import concourse.mybir as mybir
import concourse.tile as tile
from concourse import bacc

D = 128
N = 4096
B = 4
NCORES = 8
HALF = N // 2
RT = HALF // 128  # 16 row tiles per core

F32 = mybir.dt.float32
BF16 = mybir.dt.bfloat16
NPBF = ml_dtypes.bfloat16
AF = mybir.ActivationFunctionType
ALU = mybir.AluOpType


def build_nc():
    nc = bacc.Bacc()
    xt = nc.dram_tensor("xt", [8, D, 512], BF16, kind="ExternalInput")  # x[b].T pieces
    wqT = nc.dram_tensor("wqT", [D, D], BF16, kind="ExternalInput")
    wkT = nc.dram_tensor("wkT", [D, D], BF16, kind="ExternalInput")
    wvT = nc.dram_tensor("wvT", [D, D], BF16, kind="ExternalInput")
    bq = nc.dram_tensor("bq", [D, 1], F32, kind="ExternalInput")
    bk = nc.dram_tensor("bk", [D, 1], F32, kind="ExternalInput")
    out = nc.dram_tensor("out", [4, 32], F32, kind="ExternalOutput")

    with tile.TileContext(nc) as tc:
        with (
            tc.tile_pool(name="singles", bufs=1) as singles,
            tc.tile_pool(name="pp", bufs=1, space="PSUM") as pp,
            tc.tile_pool(name="epool", bufs=3) as epool,
        ):
            L = pp.tile([128, 2048], F32, tag="L", name="L")
            R = pp.tile([128, 2048], F32, tag="R", name="R")

            wq_sb = singles.tile([D, D], BF16, tag="wq", name="wq_sb")
            wk_sb = singles.tile([D, D], BF16, tag="wk", name="wk_sb")
            wv_sb = singles.tile([D, D], BF16, tag="wv", name="wv_sb")
            bqs = singles.tile([D, 1], F32, tag="bq", name="bqs")
            bks = singles.tile([D, 1], F32, tag="bk", name="bks")
            ones_sb = singles.tile([D, D], BF16, tag="ones", name="ones_sb")
            tl_out = singles.tile([D, 1], F32, tag="tl", name="tl_out")
            xt_sb = singles.tile([D, N], BF16, tag="xt", name="xt_sb")
            kt_sb = singles.tile([D, N], BF16, tag="kt", name="kt_sb")
            qt_sb = singles.tile([D, HALF], BF16, tag="qt", name="qt_sb")
            part = singles.tile([128, 2 * RT], F32, tag="part", name="part")
            Rcol = singles.tile([128, RT], F32, tag="R", name="Rcol")
            rr = singles.tile([128, RT], F32, tag="rr", name="rr")
            rmat = singles.tile([128, 2, 4, D], BF16, tag="rmat", name="rmat")
            # per-tile w slots (bf16) cast off PSUM; GPSIMD folds them into wacc
            wstore = singles.tile([128, 4, 512], BF16, tag="wst", name="wstore")
            wacc = singles.tile([128, 1024], BF16, tag="wacc", name="wacc")
            escr = singles.tile([128, 2048], F32, tag="escr", name="escr")
            vsb = singles.tile([128, 2048], BF16, tag="vsb", name="vsb")
            odump = singles.tile([128, 1024], F32, tag="odump", name="odump")
            opart = singles.tile([128, 4], F32, tag="opart", name="opart")
            o1 = singles.tile([128, 1], F32, tag="o1", name="o1")
            o128 = singles.tile([128, 32], F32, tag="o128", name="o128")
            o4x32 = singles.tile([128, 32], F32, tag="o4x32", name="o4x32")

            nc.vector.memset(ones_sb, 1.0)
            nc.vector.memset(o128, 0.0)
            nc.vector.memset(rmat, 0.0)
            nc.gpsimd.memset(wacc, 0.0)

            # ---- DMAs: parallel issue across sync / scalar / gpsimd ----
            nc.sync.dma_start(wk_sb, wkT[:, :])
            for c in (0, 3, 6):
                nc.sync.dma_start(xt_sb[:, c * 512 : (c + 1) * 512], xt[c, :, :])
            for c in (1, 4, 7):
                nc.gpsimd.dma_start(xt_sb[:, c * 512 : (c + 1) * 512], xt[c, :, :])
            nc.scalar.dma_start(wq_sb, wqT[:, :])
            for c in (2, 5):
                nc.scalar.dma_start(xt_sb[:, c * 512 : (c + 1) * 512], xt[c, :, :])
            nc.scalar.dma_start(bks, bk[:, :])
            nc.scalar.dma_start(bqs, bq[:, :])
            nc.scalar.dma_start(wv_sb, wvT[:, :])

            # early exp table load (~2.7us) while DMAs land
            nc.scalar.activation(out=tl_out, in_=ones_sb[:, 0:1], func=AF.Exp)

            # PE HAM warm-up (keeps clock at 2.4GHz through the prologue)
            for _ in range(36):
                nc.tensor.matmul(R[:, 1024:1152], ones_sb, ones_sb, start=True, stop=True)

            # ---- projections (1024-wide matmuls, bias-add drains on DVE) ----
            for g in range(4):  # K left -> L
                nc.tensor.matmul(
                    L[:, g * 512 : (g + 1) * 512],
                    wk_sb,
                    xt_sb[:, g * 512 : (g + 1) * 512],
                    start=True,
                    stop=True,
                )
            for g in range(4):  # Q (this core's rows = xt cols 0..2047) -> R
                nc.tensor.matmul(
                    R[:, g * 512 : (g + 1) * 512],
                    wq_sb,
                    xt_sb[:, g * 512 : (g + 1) * 512],
                    start=True,
                    stop=True,
                )
            for g in range(4):
                nc.vector.tensor_scalar_add(
                    out=kt_sb[:, g * 512 : (g + 1) * 512],
                    in0=L[:, g * 512 : (g + 1) * 512],
                    scalar1=bks,
                )
            nc.vector.tensor_scalar_add(out=qt_sb[:, 0:128], in0=R[:, 0:128], scalar1=bqs)
            nc.vector.tensor_scalar_add(out=qt_sb[:, 128:2048], in0=R[:, 128:2048], scalar1=bqs)
            for g in range(4):  # K right -> R (after qt drained)
                nc.tensor.matmul(
                    R[:, g * 512 : (g + 1) * 512],
                    wk_sb,
                    xt_sb[:, 2048 + g * 512 : 2048 + (g + 1) * 512],
                    start=True,
                    stop=True,
                )
            for g in range(2):
                nc.vector.tensor_scalar_add(
                    out=kt_sb[:, 2048 + g * 1024 : 2048 + (g + 1) * 1024],
                    in0=R[:, g * 1024 : (g + 1) * 1024],
                    scalar1=bks,
                )

            E_tiles = {}

            def emit_S(i, half):
                # g1-g3 first (their only dep is the previous exp read); g0 is
                # emitted as a separate batch so the w-cast wait lands on it
                # alone and banks 1-3 stage concurrently with the cast.
                reg = L if half == 0 else R
                lhsT = qt_sb[:, i * 128 : (i + 1) * 128]
                for g in (1, 2, 3):
                    nc.tensor.matmul(
                        reg[:, g * 512 : (g + 1) * 512],
                        lhsT,
                        kt_sb[:, half * 2048 + g * 512 : half * 2048 + (g + 1) * 512],
                        start=True,
                        stop=True,
                    )
                nc.tensor.matmul(
                    reg[:, 0:512],
                    lhsT,
                    kt_sb[:, half * 2048 : half * 2048 + 512],
                    start=True,
                    stop=True,
                )

            def emit_exp(i, half):
                if i not in E_tiles:
                    E_tiles[i] = epool.tile([128, N], BF16, tag="E", name=f"E_{i}")
                reg = L if half == 0 else R
                nc.scalar.activation(
                    out=E_tiles[i][:, half * 2048 : (half + 1) * 2048],
                    in_=reg,
                    func=AF.Exp,
                    accum_out=part[:, 2 * i + half : 2 * i + half + 1],
                )

            def emit_rr(i):
                nc.vector.tensor_add(
                    out=Rcol[:, i : i + 1],
                    in0=part[:, 2 * i : 2 * i + 1],
                    in1=part[:, 2 * i + 1 : 2 * i + 2],
                )
                nc.vector.reciprocal(out=rr[:, i : i + 1], in_=Rcol[:, i : i + 1])
                p = i % 2
                for j in range(4):
                    nc.gpsimd.tensor_copy(
                        out=rmat[:, p, j, 32 * j : 32 * j + 1], in_=rr[:, i : i + 1]
                    )

            def emit_w(i, half, home=0, accum_dve=False):
                """Rank-1 contraction of E tile i, m-half `half`, into the
                512-col bank at `home` of L (half 0) / R (half 1); chunk j
                lands on partition 32j.  CAST (not RMW) to a bf16 slot on
                DVE; the wacc accumulate runs on GPSIMD (or DVE in the tail
                so the epilogue is not gated on the slow GPSIMD queue)."""
                reg = L if half == 0 else R
                p = i % 2
                E = E_tiles[i]
                for j in range(4):
                    m0 = half * 2048 + j * 512
                    nc.tensor.matmul(
                        reg[:, home : home + 512],
                        rmat[:, p, j, :],
                        E[:, m0 : m0 + 512],
                        start=(j == 0),
                        stop=(j == 3),
                        skip_group_check=True,
                    )
                slot = (2 * i + half) % 4
                nc.vector.tensor_copy(out=wstore[:, slot, :], in_=reg[:, home : home + 512])
                eng = nc.vector if accum_dve else nc.gpsimd
                eng.tensor_add(
                    out=wacc[:, half * 512 : (half + 1) * 512],
                    in0=wacc[:, half * 512 : (half + 1) * 512],
                    in1=wstore[:, slot, :],
                )

            # ---- prologue: first tile ----
            emit_S(0, 0)
            emit_exp(0, 0)
            emit_S(0, 1)
            emit_exp(0, 1)

            # ---- main loop ----
            # Steady state: during exp(i,1) PE runs w(i-1,0)+S(i+1,0); during
            # exp(i+1,0) PE runs w(i-1,1)+S(i+1,1).  The L-chain after
            # exp_L(i) is w(4 MM) -> cast(DVE) -> S(2 MM) -> exp_L(i+1).
            for i in range(RT - 1):
                emit_rr(i)
                if i + 1 < RT:
                    emit_S(i + 1, 0)
                    emit_exp(i + 1, 0)
                if i >= 1:
                    emit_w(i - 1, 1)
                if i + 1 < RT:
                    emit_S(i + 1, 1)
                    emit_exp(i + 1, 1)
                emit_w(i, 0)
            # tail: w(15,0) first (its exp finished earlier), distinct psum
            # homes so the three groups don't serialize, accumulate on DVE
            emit_rr(RT - 1)
            emit_w(RT - 1, 0, home=0, accum_dve=True)
            emit_w(RT - 2, 1, home=0, accum_dve=True)
            emit_w(RT - 1, 1, home=512, accum_dve=True)

            # ---- epilogue: V fused; out[d] = (1/N) sum_m w[m] V0[m,d] ----
            for c in range(4):  # 1024-col m chunks
                hf, jb = c // 2, 2 * (c % 2)
                vreg = L[:, (c % 2) * 1024 : (c % 2 + 1) * 1024]
                for g in range(2):  # V chunk from xt (no bias; host adds bv)
                    nc.tensor.matmul(
                        vreg[:, g * 512 : (g + 1) * 512],
                        wv_sb,
                        xt_sb[:, c * 1024 + g * 512 : c * 1024 + (g + 1) * 512],
                        start=True,
                        stop=True,
                    )
                for jj in (jb, jb + 1):
                    nc.tensor.matmul(  # replicate w segment to all partitions
                        R[:, (c % 2) * 1024 + (jj - jb) * 512 : (c % 2) * 1024 + (jj - jb + 1) * 512],
                        ones_sb[32 * jj : 32 * jj + 1, :],
                        wacc[32 * jj : 32 * jj + 1, hf * 512 : (hf + 1) * 512],
                        start=True,
                        stop=True,
                        tile_position=(32 * jj, 0),
                    )
                scr = escr[:, (c % 2) * 1024 : (c % 2 + 1) * 1024]
                vch = vsb[:, (c % 2) * 1024 : (c % 2 + 1) * 1024]
                # DVE can read only one PSUM operand: stage V in SBUF first.
                # The copy runs on ACT (idle after the last exp) so DVE only
                # carries the mult+reduce chain.
                nc.scalar.activation(out=vch, in_=vreg, func=AF.Identity)
                nc.vector.tensor_mul(
                    out=scr, in0=vch, in1=R[:, (c % 2) * 1024 : (c % 2 + 1) * 1024]
                )
                if c % 2 == 0:
                    nc.scalar.activation(
                        out=odump, in_=scr, func=AF.Identity, accum_out=opart[:, c : c + 1]
                    )
                else:
                    nc.vector.tensor_reduce(
                        out=opart[:, c : c + 1], in_=scr, axis=mybir.AxisListType.X, op=ALU.add
                    )
            nc.vector.tensor_reduce(out=o1, in_=opart, axis=mybir.AxisListType.X, op=ALU.add)
            nc.scalar.activation(out=o128[:, 0:1], in_=o1, func=AF.Identity, scale=1.0 / N)
            # pack [128,1] -> rows {0,32,64,96} x 32 cols via 32x32 block
            # transpose so the output DMA moves 4 contiguous 128B rows
            # instead of 128 strided 4B elements (saves ~7us of DMA/teardown)
            nc.vector.transpose(out=o4x32, in_=o128)
            for b in range(4):
                nc.sync.dma_start(out[b : b + 1, :], o4x32[32 * b : 32 * b + 1, :])

    nc.compile()
    return nc


_cache = {}


def get_nc():
    if "nc" not in _cache:
        _cache["nc"] = build_nc()
    return _cache["nc"]


def make_in_maps(x, Wq, bq, Wk, bk, Wv, bv):
    x = np.asarray(x, np.float32)
    wqT = np.ascontiguousarray(np.asarray(Wq, np.float32).T.astype(NPBF))
    wkT = np.ascontiguousarray(np.asarray(Wk, np.float32).T.astype(NPBF))
    wvT = np.ascontiguousarray(np.asarray(Wv, np.float32).T.astype(NPBF))
    bqc = np.ascontiguousarray(np.asarray(bq, np.float32).reshape(D, 1))
    bkc = np.ascontiguousarray(np.asarray(bk, np.float32).reshape(D, 1))
    in_maps = []
    for c in range(NCORES):
        b = c // 2
        h = c % 2
        xbT = x[b].T.astype(NPBF)  # [128, 4096]
        xperm = np.concatenate(
            [xbT[:, h * HALF : (h + 1) * HALF], xbT[:, (1 - h) * HALF : (2 - h) * HALF]], axis=1
        )
        # contiguous 512-col DMA pieces: [8, 128, 512]
        xp = np.ascontiguousarray(xperm.reshape(D, 8, 512).transpose(1, 0, 2))
        in_maps.append(
            {"xt": xp, "wqT": wqT, "wkT": wkT, "wvT": wvT, "bq": bqc, "bk": bkc}
        )
    return in_maps


def combine(results, bv):
    outs = [np.asarray(results[c]["out"]).reshape(D) for c in range(NCORES)]
    bvf = np.asarray(bv, np.float32).reshape(D)
    return np.stack([outs[2 * b] + outs[2 * b + 1] + bvf for b in range(B)]).astype(np.float32)


def run(inputs, trace=False, **kwargs):
    from concourse.bass_utils import run_bass_kernel_spmd

    nc = get_nc()
    in_maps = make_in_maps(**inputs)
    res = run_bass_kernel_spmd(nc, in_maps, core_ids=list(range(NCORES)), trace=trace, **kwargs)
    return combine(res.results, inputs["bv"]), res


def kernel(x, Wq, bq, Wk, bk, Wv, bv):
    out, _ = run(dict(x=x, Wq=Wq, bq=bq, Wk=Wk, bk=bk, Wv=Wv, bv=bv))
    return out


# revision 16
# speedup vs baseline: 1.0835x; 1.0835x over previous
"""AttentionAggregation kernel for 8 TRN2 NeuronCores (v6).

Math: out[b] = mean_n softmax(Q K^T)[n,:] @ V  with Q/K/V = x @ W^T + b.
Fold: out[b,d] = sum_m w[b,m] V[b,m,d],  w[b,m] = (1/N) sum_n exp(S[n,m])/R[n],
R[n] = sum_m exp(S[n,m]).  attn@V collapses to rank-1 matmuls (r^T @ E) plus a
single weighted reduction against V.  Softmax max-subtraction skipped (|S|<~25).

Sharding: core c -> batch b=c//2, softmax-row half h=c%2 (2048 rows each).
Host permutes x[b].T columns so each core's own row-half comes first (m axis
consistently permuted; softmax/final sum are permutation-invariant).  Host
sums the two per-core partials and adds bv (exact: per-core sum_m w[m]=0.5).

v6 structure (trace-driven):
- PSUM split into FOUR independent [128,1024] tiles L1/L2/R1/R2 and each
  half-tile exp split into two 1024-wide ACTIVATEs.  Tile-framework deps are
  PSUM-tile granular, so with 2048-col regions the chain
  exp -> w-matmuls -> cast -> S-staging serialized at 5.56us/tile.  With
  four tiles the w+cast only block L1 while L2/R2 staging and the other
  exps proceed -> the chain hides under ~4us of ACT queue.
- ACT pace: 4 x (1024+352)/1.2 + 2 READ_ACC = 5.16us/tile.  Row sums R come
  from accum_out on the p2 exps plus DVE tensor_reduce of the p1 E-quarters.
- w psum is CAST to a bf16 slot (DVE); accumulation into wacc on GPSIMD
  (SBUF-only engine!), rmat copies on GPSIMD; last-tile adds on DVE so the
  epilogue is not gated on the GPSIMD queue.
- V projection fused into the epilogue; V-casts on ACT (idle there), mults
  on DVE.  Output packed via 32x32 block transpose into 4 contiguous rows
  (a [128,1] strided out-DMA costs ~7us in ring/teardown time).
- x arrives as 8 contiguous [128,512] pieces (1KB DMA elements) issued from
  sync+gpsimd queues; weights/biases on the scalar queue.  PE HAM warm-up
  dummies + early exp-table load run during the DMA wait.

HW notes: bf16 everywhere PE-facing; matmul out <= 512 fp32 cols (one PSUM
bank); GPSIMD cannot touch PSUM; DVE reads at most one PSUM operand;
tensor_tensor_reduce faults on HW.
"""

import sys

sys.path.insert(0, "/opt/trn_rl_repo")

import ml_dtypes
import numpy as np

import concourse.bass as bass
import concourse.mybir as mybir
import concourse.tile as tile
from concourse import bacc

D = 128
N = 4096
B = 4
NCORES = 8
HALF = N // 2
RT = HALF // 128  # 16 row tiles per core

F32 = mybir.dt.float32
BF16 = mybir.dt.bfloat16
NPBF = ml_dtypes.bfloat16
AF = mybir.ActivationFunctionType
ALU = mybir.AluOpType


def build_nc():
    nc = bacc.Bacc()
    xt = nc.dram_tensor("xt", [8, D, 512], BF16, kind="ExternalInput")
    wqT = nc.dram_tensor("wqT", [D, D], BF16, kind="ExternalInput")
    wkT = nc.dram_tensor("wkT", [D, D], BF16, kind="ExternalInput")
    wvT = nc.dram_tensor("wvT", [D, D], BF16, kind="ExternalInput")
    bq = nc.dram_tensor("bq", [D, 1], F32, kind="ExternalInput")
    bk = nc.dram_tensor("bk", [D, 1], F32, kind="ExternalInput")
    out = nc.dram_tensor("out", [4, 32], F32, kind="ExternalOutput")

    with tile.TileContext(nc) as tc:
        with (
            tc.tile_pool(name="singles", bufs=1) as singles,
            tc.tile_pool(name="pp", bufs=1, space="PSUM") as pp,
            tc.tile_pool(name="epool", bufs=3) as epool,
        ):
            L1 = pp.tile([128, 1024], F32, tag="L1", name="L1")
            L2 = pp.tile([128, 1024], F32, tag="L2", name="L2")
            R1 = pp.tile([128, 1024], F32, tag="R1", name="R1")
            R2 = pp.tile([128, 1024], F32, tag="R2", name="R2")
            P1 = {0: L1, 1: R1}
            P2 = {0: L2, 1: R2}

            wq_sb = singles.tile([D, D], BF16, tag="wq", name="wq_sb")
            wk_sb = singles.tile([D, D], BF16, tag="wk", name="wk_sb")
            wv_sb = singles.tile([D, D], BF16, tag="wv", name="wv_sb")
            bqs = singles.tile([D, 1], F32, tag="bq", name="bqs")
            bks = singles.tile([D, 1], F32, tag="bk", name="bks")
            ones_sb = singles.tile([D, D], BF16, tag="ones", name="ones_sb")
            tl_out = singles.tile([D, 1], F32, tag="tl", name="tl_out")
            xt_sb = singles.tile([D, N], BF16, tag="xt", name="xt_sb")
            kt_sb = singles.tile([D, N], BF16, tag="kt", name="kt_sb")
            qt_sb = singles.tile([D, HALF], BF16, tag="qt", name="qt_sb")
            part = singles.tile([128, 4 * RT], F32, tag="part", name="part")
            Rcol = singles.tile([128, RT], F32, tag="R", name="Rcol")
            rr = singles.tile([128, RT], F32, tag="rr", name="rr")
            rmat = singles.tile([128, 2, 4, D], BF16, tag="rmat", name="rmat")
            wstore = singles.tile([128, 4, 512], BF16, tag="wst", name="wstore")
            wacc = singles.tile([128, 1024], BF16, tag="wacc", name="wacc")
            escr = singles.tile([128, 2048], F32, tag="escr", name="escr")
            vsb = singles.tile([128, 2048], BF16, tag="vsb", name="vsb")
            odump = singles.tile([128, 1024], F32, tag="odump", name="odump")
            opart = singles.tile([128, 4], F32, tag="opart", name="opart")
            o1 = singles.tile([128, 1], F32, tag="o1", name="o1")
            o128 = singles.tile([128, 32], F32, tag="o128", name="o128")
            o4x32 = singles.tile([128, 32], F32, tag="o4x32", name="o4x32")

            nc.vector.memset(ones_sb, 1.0)
            nc.vector.memset(o128, 0.0)
            nc.vector.memset(rmat, 0.0)
            nc.gpsimd.memset(wacc, 0.0)

            # ---- DMAs: contiguous 512-col pieces, sync+gpsimd queues ----
            nc.sync.dma_start(wk_sb, wkT[:, :])
            for c in (0, 2, 4, 6):
                nc.sync.dma_start(xt_sb[:, c * 512 : (c + 1) * 512], xt[c, :, :])
            for c in (1, 3, 5, 7):
                nc.gpsimd.dma_start(xt_sb[:, c * 512 : (c + 1) * 512], xt[c, :, :])
            nc.scalar.dma_start(wq_sb, wqT[:, :])
            nc.scalar.dma_start(bks, bk[:, :])
            nc.scalar.dma_start(bqs, bq[:, :])
            nc.scalar.dma_start(wv_sb, wvT[:, :])

            # early exp table load while DMAs land
            nc.scalar.activation(out=tl_out, in_=ones_sb[:, 0:1], func=AF.Exp)

            # PE HAM warm-up
            for _ in range(36):
                nc.tensor.matmul(R2[:, 0:128], ones_sb, ones_sb, start=True, stop=True)

            # ---- projections ----
            def proj(w_sb, src0, dst_lo, dst_hi):
                for g in range(2):
                    nc.tensor.matmul(
                        dst_lo[:, g * 512 : (g + 1) * 512],
                        w_sb,
                        xt_sb[:, src0 + g * 512 : src0 + (g + 1) * 512],
                        start=True,
                        stop=True,
                    )
                for g in range(2):
                    nc.tensor.matmul(
                        dst_hi[:, g * 512 : (g + 1) * 512],
                        w_sb,
                        xt_sb[:, src0 + 1024 + g * 512 : src0 + 1024 + (g + 1) * 512],
                        start=True,
                        stop=True,
                    )

            proj(wk_sb, 0, L1, L2)  # K left
            proj(wq_sb, 0, R1, R2)  # Q
            nc.vector.tensor_scalar_add(out=kt_sb[:, 0:1024], in0=L1, scalar1=bks)
            nc.vector.tensor_scalar_add(out=kt_sb[:, 1024:2048], in0=L2, scalar1=bks)
            nc.vector.tensor_scalar_add(out=qt_sb[:, 0:128], in0=R1[:, 0:128], scalar1=bqs)
            nc.vector.tensor_scalar_add(out=qt_sb[:, 128:1024], in0=R1[:, 128:1024], scalar1=bqs)
            nc.vector.tensor_scalar_add(out=qt_sb[:, 1024:2048], in0=R2, scalar1=bqs)
            proj(wk_sb, 2048, R1, R2)  # K right (after qt drained)
            nc.vector.tensor_scalar_add(out=kt_sb[:, 2048:3072], in0=R1, scalar1=bks)
            nc.vector.tensor_scalar_add(out=kt_sb[:, 3072:4096], in0=R2, scalar1=bks)

            E_tiles = {}

            def get_E(i):
                if i not in E_tiles:
                    E_tiles[i] = epool.tile([128, N], BF16, tag="E", name=f"E_{i}")
                return E_tiles[i]

            def emit_S(i, half, part_idx):
                """Stage S cols for row tile i into (L|R){1,2}."""
                reg = (P1 if part_idx == 1 else P2)[half]
                base = half * 2048 + (part_idx - 1) * 1024
                lhsT = qt_sb[:, i * 128 : (i + 1) * 128]
                for g in range(2):
                    nc.tensor.matmul(
                        reg[:, g * 512 : (g + 1) * 512],
                        lhsT,
                        kt_sb[:, base + g * 512 : base + (g + 1) * 512],
                        start=True,
                        stop=True,
                    )

            def emit_exp(i, half, part_idx):
                """1024-wide exp; the p2 parts carry the free accum_out."""
                reg = (P1 if part_idx == 1 else P2)[half]
                base = half * 2048 + (part_idx - 1) * 1024
                acc = part[:, 4 * i + half : 4 * i + half + 1] if part_idx == 2 else None
                nc.scalar.activation(
                    out=get_E(i)[:, base : base + 1024],
                    in_=reg,
                    func=AF.Exp,
                    accum_out=acc,
                )

            def emit_tr(i, half):
                """DVE row-sum of the p1 E-quarter (no ACT READ_ACC cost)."""
                base = half * 2048
                nc.vector.tensor_reduce(
                    out=part[:, 4 * i + 2 + half : 4 * i + 3 + half],
                    in_=get_E(i)[:, base : base + 1024],
                    axis=mybir.AxisListType.X,
                    op=ALU.add,
                )

            def emit_rr(i):
                nc.vector.tensor_reduce(
                    out=Rcol[:, i : i + 1],
                    in_=part[:, 4 * i : 4 * i + 4],
                    axis=mybir.AxisListType.X,
                    op=ALU.add,
                )
                nc.vector.reciprocal(out=rr[:, i : i + 1], in_=Rcol[:, i : i + 1])
                p = i % 2
                for j in range(4):
                    nc.gpsimd.tensor_copy(
                        out=rmat[:, p, j, 32 * j : 32 * j + 1], in_=rr[:, i : i + 1]
                    )

            def emit_w(i, half, accum_dve=False):
                """Rank-1 contraction of E tile i, m-half `half`, into
                (L1|R1)[:, 0:512]; chunk j lands on partition 32j; CAST to a
                bf16 slot on DVE; accumulate on GPSIMD (DVE in the tail)."""
                reg = P1[half]
                p = i % 2
                E = E_tiles[i]
                for j in range(4):
                    m0 = half * 2048 + j * 512
                    nc.tensor.matmul(
                        reg[:, 0:512],
                        rmat[:, p, j, :],
                        E[:, m0 : m0 + 512],
                        start=(j == 0),
                        stop=(j == 3),
                        skip_group_check=True,
                    )
                slot = (2 * i + half) % 4
                nc.vector.tensor_copy(out=wstore[:, slot, :], in_=reg[:, 0:512])
                eng = nc.vector if accum_dve else nc.gpsimd
                eng.tensor_add(
                    out=wacc[:, half * 512 : (half + 1) * 512],
                    in0=wacc[:, half * 512 : (half + 1) * 512],
                    in1=wstore[:, slot, :],
                )

            # ---- prologue: first tile ----
            emit_S(0, 0, 1)
            emit_S(0, 0, 2)
            emit_exp(0, 0, 1)
            emit_exp(0, 0, 2)
            emit_S(0, 1, 1)
            emit_S(0, 1, 2)
            emit_exp(0, 1, 1)
            emit_exp(0, 1, 2)

            # ---- main loop ----
            # L1-chain per tile: expL(i)p1 -> w(i-1,0) -> cast -> S(i+1,0)p1
            # hides under the ~4us of remaining ACT work before expL(i+1)p1.
            for i in range(RT):
                last = i + 1 >= RT
                if i >= 1:
                    emit_w(i - 1, 0, accum_dve=last)
                if not last:
                    emit_S(i + 1, 0, 1)
                    emit_S(i + 1, 0, 2)
                    emit_exp(i + 1, 0, 1)
                    emit_exp(i + 1, 0, 2)
                emit_tr(i, 0)
                if i >= 1:
                    emit_w(i - 1, 1, accum_dve=last)
                if not last:
                    emit_S(i + 1, 1, 1)
                    emit_S(i + 1, 1, 2)
                    emit_exp(i + 1, 1, 1)
                    emit_exp(i + 1, 1, 2)
                emit_tr(i, 1)
                emit_rr(i)
            emit_w(RT - 1, 0, accum_dve=True)
            emit_w(RT - 1, 1, accum_dve=True)

            # ---- epilogue: V fused; out[d] = (1/N) sum_m w[m] V0[m,d] ----
            for c in range(4):
                hf, jb = c // 2, 2 * (c % 2)
                vreg = L1 if c % 2 == 0 else L2
                rreg = R1 if c % 2 == 0 else R2
                for g in range(2):
                    nc.tensor.matmul(
                        vreg[:, g * 512 : (g + 1) * 512],
                        wv_sb,
                        xt_sb[:, c * 1024 + g * 512 : c * 1024 + (g + 1) * 512],
                        start=True,
                        stop=True,
                    )
                for jj in (jb, jb + 1):
                    nc.tensor.matmul(
                        rreg[:, (jj - jb) * 512 : (jj - jb + 1) * 512],
                        ones_sb[32 * jj : 32 * jj + 1, :],
                        wacc[32 * jj : 32 * jj + 1, hf * 512 : (hf + 1) * 512],
                        start=True,
                        stop=True,
                        tile_position=(32 * jj, 0),
                    )
                scr = escr[:, (c % 2) * 1024 : (c % 2 + 1) * 1024]
                vch = vsb[:, (c % 2) * 1024 : (c % 2 + 1) * 1024]
                # stage V in SBUF via ACT (idle after last exp); mult on DVE
                nc.scalar.activation(out=vch, in_=vreg, func=AF.Identity)
                nc.vector.tensor_mul(out=scr, in0=vch, in1=rreg)
                if c % 2 == 0:
                    nc.scalar.activation(
                        out=odump, in_=scr, func=AF.Identity, accum_out=opart[:, c : c + 1]
                    )
                else:
                    nc.vector.tensor_reduce(
                        out=opart[:, c : c + 1], in_=scr, axis=mybir.AxisListType.X, op=ALU.add
                    )
            nc.vector.tensor_reduce(out=o1, in_=opart, axis=mybir.AxisListType.X, op=ALU.add)
            nc.scalar.activation(out=o128[:, 0:1], in_=o1, func=AF.Identity, scale=1.0 / N)
            # pack to 4 contiguous 128B rows for a fast output DMA
            nc.vector.transpose(out=o4x32, in_=o128)
            for b in range(4):
                nc.sync.dma_start(out[b : b + 1, :], o4x32[32 * b : 32 * b + 1, :])

    nc.compile()
    return nc


_cache = {}


def get_nc():
    if "nc" not in _cache:
        _cache["nc"] = build_nc()
    return _cache["nc"]


def make_in_maps(x, Wq, bq, Wk, bk, Wv, bv):
    x = np.asarray(x, np.float32)
    wqT = np.ascontiguousarray(np.asarray(Wq, np.float32).T.astype(NPBF))
    wkT = np.ascontiguousarray(np.asarray(Wk, np.float32).T.astype(NPBF))
    wvT = np.ascontiguousarray(np.asarray(Wv, np.float32).T.astype(NPBF))
    bqc = np.ascontiguousarray(np.asarray(bq, np.float32).reshape(D, 1))
    bkc = np.ascontiguousarray(np.asarray(bk, np.float32).reshape(D, 1))
    in_maps = []
    for c in range(NCORES):
        b = c // 2
        h = c % 2
        xbT = x[b].T.astype(NPBF)
        xperm = np.concatenate(
            [xbT[:, h * HALF : (h + 1) * HALF], xbT[:, (1 - h) * HALF : (2 - h) * HALF]], axis=1
        )
        xp = np.ascontiguousarray(xperm.reshape(D, 8, 512).transpose(1, 0, 2))
        in_maps.append(
            {"xt": xp, "wqT": wqT, "wkT": wkT, "wvT": wvT, "bq": bqc, "bk": bkc}
        )
    return in_maps


def combine(results, bv):
    outs = [np.asarray(results[c]["out"]).reshape(D) for c in range(NCORES)]
    bvf = np.asarray(bv, np.float32).reshape(D)
    return np.stack([outs[2 * b] + outs[2 * b + 1] + bvf for b in range(B)]).astype(np.float32)


def run(inputs, trace=False, **kwargs):
    from concourse.bass_utils import run_bass_kernel_spmd

    nc = get_nc()
    in_maps = make_in_maps(**inputs)
    res = run_bass_kernel_spmd(nc, in_maps, core_ids=list(range(NCORES)), trace=trace, **kwargs)
    return combine(res.results, inputs["bv"]), res


def kernel(x, Wq, bq, Wk, bk, Wv, bv):
    out, _ = run(dict(x=x, Wq=Wq, bq=bq, Wk=Wk, bk=bk, Wv=Wv, bv=bv))
    return out


# revision 18
# speedup vs baseline: 1.0915x; 1.0074x over previous
"""AttentionAggregation kernel for 8 TRN2 NeuronCores (v6).

Math: out[b] = mean_n softmax(Q K^T)[n,:] @ V  with Q/K/V = x @ W^T + b.
Fold: out[b,d] = sum_m w[b,m] V[b,m,d],  w[b,m] = (1/N) sum_n exp(S[n,m])/R[n],
R[n] = sum_m exp(S[n,m]).  attn@V collapses to rank-1 matmuls (r^T @ E) plus a
single weighted reduction against V.  Softmax max-subtraction skipped (|S|<~25).

Sharding: core c -> batch b=c//2, softmax-row half h=c%2 (2048 rows each).
Host permutes x[b].T columns so each core's own row-half comes first (m axis
consistently permuted; softmax/final sum are permutation-invariant).  Host
sums the two per-core partials and adds bv (exact: per-core sum_m w[m]=0.5).

v6 structure (trace-driven):
- PSUM split into FOUR independent [128,1024] tiles L1/L2/R1/R2 and each
  half-tile exp split into two 1024-wide ACTIVATEs.  Tile-framework deps are
  PSUM-tile granular, so with 2048-col regions the chain
  exp -> w-matmuls -> cast -> S-staging serialized at 5.56us/tile.  With
  four tiles the w+cast only block L1 while L2/R2 staging and the other
  exps proceed -> the chain hides under ~4us of ACT queue.
- ACT pace: 4 x (1024+352)/1.2 + 2 READ_ACC = 5.16us/tile.  Row sums R come
  from accum_out on the p2 exps plus DVE tensor_reduce of the p1 E-quarters.
- w psum is CAST to a bf16 slot (DVE); accumulation into wacc on GPSIMD
  (SBUF-only engine!), rmat copies on GPSIMD; last-tile adds on DVE so the
  epilogue is not gated on the GPSIMD queue.
- V projection fused into the epilogue; V-casts on ACT (idle there), mults
  on DVE.  Output packed via 32x32 block transpose into 4 contiguous rows
  (a [128,1] strided out-DMA costs ~7us in ring/teardown time).
- x arrives as 8 contiguous [128,512] pieces (1KB DMA elements) issued from
  sync+gpsimd queues; weights/biases on the scalar queue.  PE HAM warm-up
  dummies + early exp-table load run during the DMA wait.

HW notes: bf16 everywhere PE-facing; matmul out <= 512 fp32 cols (one PSUM
bank); GPSIMD cannot touch PSUM; DVE reads at most one PSUM operand;
tensor_tensor_reduce faults on HW.
"""

import sys

sys.path.insert(0, "/opt/trn_rl_repo")

import ml_dtypes
import numpy as np

import concourse.bass as bass
import concourse.mybir as mybir
import concourse.tile as tile
from concourse import bacc

D = 128
N = 4096
B = 4
NCORES = 8
HALF = N // 2
RT = HALF // 128  # 16 row tiles per core

F32 = mybir.dt.float32
BF16 = mybir.dt.bfloat16
NPBF = ml_dtypes.bfloat16
AF = mybir.ActivationFunctionType
ALU = mybir.AluOpType


def build_nc():
    nc = bacc.Bacc()
    xt = nc.dram_tensor("xt", [8, D, 512], BF16, kind="ExternalInput")
    wqT = nc.dram_tensor("wqT", [D, D], BF16, kind="ExternalInput")
    wkT = nc.dram_tensor("wkT", [D, D], BF16, kind="ExternalInput")
    wvT = nc.dram_tensor("wvT", [D, D], BF16, kind="ExternalInput")
    bq = nc.dram_tensor("bq", [D, 1], F32, kind="ExternalInput")
    bk = nc.dram_tensor("bk", [D, 1], F32, kind="ExternalInput")
    out = nc.dram_tensor("out", [4, 32], F32, kind="ExternalOutput")

    with tile.TileContext(nc) as tc:
        with (
            tc.tile_pool(name="singles", bufs=1) as singles,
            tc.tile_pool(name="pp", bufs=1, space="PSUM") as pp,
            tc.tile_pool(name="epool", bufs=3) as epool,
        ):
            L1 = pp.tile([128, 1024], F32, tag="L1", name="L1")
            L2 = pp.tile([128, 1024], F32, tag="L2", name="L2")
            R1 = pp.tile([128, 1024], F32, tag="R1", name="R1")
            R2 = pp.tile([128, 1024], F32, tag="R2", name="R2")
            P1 = {0: L1, 1: R1}
            P2 = {0: L2, 1: R2}

            wq_sb = singles.tile([D, D], BF16, tag="wq", name="wq_sb")
            wk_sb = singles.tile([D, D], BF16, tag="wk", name="wk_sb")
            wv_sb = singles.tile([D, D], BF16, tag="wv", name="wv_sb")
            bqs = singles.tile([D, 1], F32, tag="bq", name="bqs")
            bks = singles.tile([D, 1], F32, tag="bk", name="bks")
            ones_sb = singles.tile([D, D], BF16, tag="ones", name="ones_sb")
            tl_out = singles.tile([D, 1], F32, tag="tl", name="tl_out")
            xt_sb = singles.tile([D, N], BF16, tag="xt", name="xt_sb")
            kt_sb = singles.tile([D, N], BF16, tag="kt", name="kt_sb")
            qt_sb = singles.tile([D, HALF], BF16, tag="qt", name="qt_sb")
            part = singles.tile([128, 4 * RT], F32, tag="part", name="part")
            Rcol = singles.tile([128, RT], F32, tag="R", name="Rcol")
            rr = singles.tile([128, RT], F32, tag="rr", name="rr")
            rmat = singles.tile([128, 2, 4, D], BF16, tag="rmat", name="rmat")
            wstore = singles.tile([128, 4, 512], BF16, tag="wst", name="wstore")
            wacc = singles.tile([128, 1024], BF16, tag="wacc", name="wacc")
            escr = singles.tile([128, 2048], F32, tag="escr", name="escr")
            vsb = singles.tile([128, 2048], BF16, tag="vsb", name="vsb")
            odump = singles.tile([128, 1024], F32, tag="odump", name="odump")
            opart = singles.tile([128, 4], F32, tag="opart", name="opart")
            o1 = singles.tile([128, 1], F32, tag="o1", name="o1")
            o128 = singles.tile([128, 32], F32, tag="o128", name="o128")
            o4x32 = singles.tile([128, 32], F32, tag="o4x32", name="o4x32")

            nc.vector.memset(ones_sb, 1.0)
            nc.vector.memset(o128, 0.0)
            nc.vector.memset(rmat, 0.0)
            nc.gpsimd.memset(wacc, 0.0)

            # ---- DMAs: contiguous 512-col pieces, sync+gpsimd queues ----
            nc.sync.dma_start(wk_sb, wkT[:, :])
            # lead pieces (cols 0-2047 gate K-left and Q) as 256-col halves
            # so they spread across more rings and arrive ~2us earlier
            for c in (0, 1, 2, 3):
                nc.sync.dma_start(
                    xt_sb[:, c * 512 : c * 512 + 256], xt[c, :, 0:256]
                )
                nc.gpsimd.dma_start(
                    xt_sb[:, c * 512 + 256 : (c + 1) * 512], xt[c, :, 256:512]
                )
            for c in (4, 6):
                nc.sync.dma_start(xt_sb[:, c * 512 : (c + 1) * 512], xt[c, :, :])
            for c in (5, 7):
                nc.gpsimd.dma_start(xt_sb[:, c * 512 : (c + 1) * 512], xt[c, :, :])
            nc.scalar.dma_start(wq_sb, wqT[:, :])
            nc.scalar.dma_start(bks, bk[:, :])
            nc.scalar.dma_start(bqs, bq[:, :])
            nc.scalar.dma_start(wv_sb, wvT[:, :])

            # early exp table load while DMAs land
            nc.scalar.activation(out=tl_out, in_=ones_sb[:, 0:1], func=AF.Exp)

            # PE HAM warm-up
            for _ in range(36):
                nc.tensor.matmul(R2[:, 0:128], ones_sb, ones_sb, start=True, stop=True)

            # ---- projections ----
            def proj(w_sb, src0, dst_lo, dst_hi):
                for g in range(2):
                    nc.tensor.matmul(
                        dst_lo[:, g * 512 : (g + 1) * 512],
                        w_sb,
                        xt_sb[:, src0 + g * 512 : src0 + (g + 1) * 512],
                        start=True,
                        stop=True,
                    )
                for g in range(2):
                    nc.tensor.matmul(
                        dst_hi[:, g * 512 : (g + 1) * 512],
                        w_sb,
                        xt_sb[:, src0 + 1024 + g * 512 : src0 + 1024 + (g + 1) * 512],
                        start=True,
                        stop=True,
                    )

            proj(wk_sb, 0, L1, L2)  # K left
            proj(wq_sb, 0, R1, R2)  # Q
            nc.vector.tensor_scalar_add(out=kt_sb[:, 0:1024], in0=L1, scalar1=bks)
            nc.vector.tensor_scalar_add(out=kt_sb[:, 1024:2048], in0=L2, scalar1=bks)
            nc.vector.tensor_scalar_add(out=qt_sb[:, 0:128], in0=R1[:, 0:128], scalar1=bqs)
            # bulk qt drains ride the idle ACT engine (free affine: in+bias)
            nc.scalar.activation(
                out=qt_sb[:, 128:1024], in_=R1[:, 128:1024], func=AF.Identity, bias=bqs
            )
            nc.scalar.activation(out=qt_sb[:, 1024:2048], in_=R2, func=AF.Identity, bias=bqs)
            proj(wk_sb, 2048, R1, R2)  # K right (after qt drained)
            nc.vector.tensor_scalar_add(out=kt_sb[:, 2048:3072], in0=R1, scalar1=bks)
            nc.vector.tensor_scalar_add(out=kt_sb[:, 3072:4096], in0=R2, scalar1=bks)

            E_tiles = {}

            def get_E(i):
                if i not in E_tiles:
                    E_tiles[i] = epool.tile([128, N], BF16, tag="E", name=f"E_{i}")
                return E_tiles[i]

            def emit_S(i, half, part_idx):
                """Stage S cols for row tile i into (L|R){1,2}."""
                reg = (P1 if part_idx == 1 else P2)[half]
                base = half * 2048 + (part_idx - 1) * 1024
                lhsT = qt_sb[:, i * 128 : (i + 1) * 128]
                for g in range(2):
                    nc.tensor.matmul(
                        reg[:, g * 512 : (g + 1) * 512],
                        lhsT,
                        kt_sb[:, base + g * 512 : base + (g + 1) * 512],
                        start=True,
                        stop=True,
                    )

            def emit_exp(i, half, part_idx):
                """1024-wide exp; the p2 parts carry the free accum_out."""
                reg = (P1 if part_idx == 1 else P2)[half]
                base = half * 2048 + (part_idx - 1) * 1024
                acc = part[:, 4 * i + half : 4 * i + half + 1] if part_idx == 2 else None
                nc.scalar.activation(
                    out=get_E(i)[:, base : base + 1024],
                    in_=reg,
                    func=AF.Exp,
                    accum_out=acc,
                )

            def emit_tr(i, half):
                """DVE row-sum of the p1 E-quarter (no ACT READ_ACC cost)."""
                base = half * 2048
                nc.vector.tensor_reduce(
                    out=part[:, 4 * i + 2 + half : 4 * i + 3 + half],
                    in_=get_E(i)[:, base : base + 1024],
                    axis=mybir.AxisListType.X,
                    op=ALU.add,
                )

            def emit_rr(i):
                nc.vector.tensor_reduce(
                    out=Rcol[:, i : i + 1],
                    in_=part[:, 4 * i : 4 * i + 4],
                    axis=mybir.AxisListType.X,
                    op=ALU.add,
                )
                nc.vector.reciprocal(out=rr[:, i : i + 1], in_=Rcol[:, i : i + 1])
                p = i % 2
                for j in range(4):
                    nc.gpsimd.tensor_copy(
                        out=rmat[:, p, j, 32 * j : 32 * j + 1], in_=rr[:, i : i + 1]
                    )

            def emit_w(i, half, accum_dve=False):
                """Rank-1 contraction of E tile i, m-half `half`, into
                (L1|R1)[:, 0:512]; chunk j lands on partition 32j; CAST to a
                bf16 slot on DVE; accumulate on GPSIMD (DVE in the tail)."""
                reg = P1[half]
                p = i % 2
                E = E_tiles[i]
                for j in range(4):
                    m0 = half * 2048 + j * 512
                    nc.tensor.matmul(
                        reg[:, 0:512],
                        rmat[:, p, j, :],
                        E[:, m0 : m0 + 512],
                        start=(j == 0),
                        stop=(j == 3),
                        skip_group_check=True,
                    )
                slot = (2 * i + half) % 4
                nc.vector.tensor_copy(out=wstore[:, slot, :], in_=reg[:, 0:512])
                eng = nc.vector if accum_dve else nc.gpsimd
                eng.tensor_add(
                    out=wacc[:, half * 512 : (half + 1) * 512],
                    in0=wacc[:, half * 512 : (half + 1) * 512],
                    in1=wstore[:, slot, :],
                )

            # ---- prologue: first tile ----
            emit_S(0, 0, 1)
            emit_S(0, 0, 2)
            emit_exp(0, 0, 1)
            emit_exp(0, 0, 2)
            emit_S(0, 1, 1)
            emit_S(0, 1, 2)
            emit_exp(0, 1, 1)
            emit_exp(0, 1, 2)

            # ---- main loop ----
            # L1-chain per tile: expL(i)p1 -> w(i-1,0) -> cast -> S(i+1,0)p1
            # hides under the ~4us of remaining ACT work before expL(i+1)p1.
            for i in range(RT):
                last = i + 1 >= RT
                if i >= 1:
                    emit_w(i - 1, 0, accum_dve=last)
                if not last:
                    emit_S(i + 1, 0, 1)
                    emit_S(i + 1, 0, 2)
                    emit_exp(i + 1, 0, 1)
                    emit_exp(i + 1, 0, 2)
                emit_tr(i, 0)
                if i >= 1:
                    emit_w(i - 1, 1, accum_dve=last)
                if not last:
                    emit_S(i + 1, 1, 1)
                    emit_S(i + 1, 1, 2)
                    emit_exp(i + 1, 1, 1)
                    emit_exp(i + 1, 1, 2)
                emit_tr(i, 1)
                emit_rr(i)
            emit_w(RT - 1, 0, accum_dve=True)
            emit_w(RT - 1, 1, accum_dve=True)

            # ---- epilogue: V fused; out[d] = (1/N) sum_m w[m] V0[m,d] ----
            # Two rounds of two chunks; within a round the ACT V-casts and
            # DVE mults stream back-to-back instead of gating per chunk.
            def epi_stage(c):
                hf, jb = c // 2, 2 * (c % 2)
                vreg = L1 if c % 2 == 0 else L2
                rreg = R1 if c % 2 == 0 else R2
                for g in range(2):
                    nc.tensor.matmul(
                        vreg[:, g * 512 : (g + 1) * 512],
                        wv_sb,
                        xt_sb[:, c * 1024 + g * 512 : c * 1024 + (g + 1) * 512],
                        start=True,
                        stop=True,
                    )
                for jj in (jb, jb + 1):
                    nc.tensor.matmul(
                        rreg[:, (jj - jb) * 512 : (jj - jb + 1) * 512],
                        ones_sb[32 * jj : 32 * jj + 1, :],
                        wacc[32 * jj : 32 * jj + 1, hf * 512 : (hf + 1) * 512],
                        start=True,
                        stop=True,
                        tile_position=(32 * jj, 0),
                    )

            def epi_castmult(c):
                half = c % 2
                nc.scalar.activation(
                    out=vsb[:, half * 1024 : (half + 1) * 1024],
                    in_=L1 if half == 0 else L2,
                    func=AF.Identity,
                )

            def epi_mult(c):
                half = c % 2
                nc.vector.tensor_mul(
                    out=escr[:, half * 1024 : (half + 1) * 1024],
                    in0=vsb[:, half * 1024 : (half + 1) * 1024],
                    in1=R1 if half == 0 else R2,
                )

            def epi_reduce(c):
                half = c % 2
                if half == 0:
                    nc.scalar.activation(
                        out=odump,
                        in_=escr[:, 0:1024],
                        func=AF.Identity,
                        accum_out=opart[:, c : c + 1],
                    )
                else:
                    nc.vector.tensor_reduce(
                        out=opart[:, c : c + 1],
                        in_=escr[:, 1024:2048],
                        axis=mybir.AxisListType.X,
                        op=ALU.add,
                    )

            epi_stage(0)
            epi_stage(1)
            epi_castmult(0)
            epi_castmult(1)
            epi_mult(0)
            epi_mult(1)
            epi_stage(2)
            epi_stage(3)
            epi_reduce(0)
            epi_reduce(1)
            epi_castmult(2)
            epi_castmult(3)
            epi_mult(2)
            epi_mult(3)
            epi_reduce(2)
            epi_reduce(3)
            nc.vector.tensor_reduce(out=o1, in_=opart, axis=mybir.AxisListType.X, op=ALU.add)
            nc.scalar.activation(out=o128[:, 0:1], in_=o1, func=AF.Identity, scale=1.0 / N)
            # pack to 4 contiguous 128B rows for a fast output DMA
            nc.vector.transpose(out=o4x32, in_=o128)
            for b in range(4):
                nc.sync.dma_start(out[b : b + 1, :], o4x32[32 * b : 32 * b + 1, :])

    nc.compile()
    return nc


_cache = {}


def get_nc():
    if "nc" not in _cache:
        _cache["nc"] = build_nc()
    return _cache["nc"]


def make_in_maps(x, Wq, bq, Wk, bk, Wv, bv):
    x = np.asarray(x, np.float32)
    wqT = np.ascontiguousarray(np.asarray(Wq, np.float32).T.astype(NPBF))
    wkT = np.ascontiguousarray(np.asarray(Wk, np.float32).T.astype(NPBF))
    wvT = np.ascontiguousarray(np.asarray(Wv, np.float32).T.astype(NPBF))
    bqc = np.ascontiguousarray(np.asarray(bq, np.float32).reshape(D, 1))
    bkc = np.ascontiguousarray(np.asarray(bk, np.float32).reshape(D, 1))
    in_maps = []
    for c in range(NCORES):
        b = c // 2
        h = c % 2
        xbT = x[b].T.astype(NPBF)
        xperm = np.concatenate(
            [xbT[:, h * HALF : (h + 1) * HALF], xbT[:, (1 - h) * HALF : (2 - h) * HALF]], axis=1
        )
        xp = np.ascontiguousarray(xperm.reshape(D, 8, 512).transpose(1, 0, 2))
        in_maps.append(
            {"xt": xp, "wqT": wqT, "wkT": wkT, "wvT": wvT, "bq": bqc, "bk": bkc}
        )
    return in_maps


def combine(results, bv):
    outs = [np.asarray(results[c]["out"]).reshape(D) for c in range(NCORES)]
    bvf = np.asarray(bv, np.float32).reshape(D)
    return np.stack([outs[2 * b] + outs[2 * b + 1] + bvf for b in range(B)]).astype(np.float32)


def run(inputs, trace=False, **kwargs):
    from concourse.bass_utils import run_bass_kernel_spmd

    nc = get_nc()
    in_maps = make_in_maps(**inputs)
    res = run_bass_kernel_spmd(nc, in_maps, core_ids=list(range(NCORES)), trace=trace, **kwargs)
    return combine(res.results, inputs["bv"]), res


def kernel(x, Wq, bq, Wk, bk, Wv, bv):
    out, _ = run(dict(x=x, Wq=Wq, bq=bq, Wk=Wk, bk=bk, Wv=Wv, bv=bv))
    return out


# revision 20
# speedup vs baseline: 1.1188x; 1.0250x over previous
"""AttentionAggregation kernel for 8 TRN2 NeuronCores (v6).

Math: out[b] = mean_n softmax(Q K^T)[n,:] @ V  with Q/K/V = x @ W^T + b.
Fold: out[b,d] = sum_m w[b,m] V[b,m,d],  w[b,m] = (1/N) sum_n exp(S[n,m])/R[n],
R[n] = sum_m exp(S[n,m]).  attn@V collapses to rank-1 matmuls (r^T @ E) plus a
single weighted reduction against V.  Softmax max-subtraction skipped (|S|<~25).

Sharding: core c -> batch b=c//2, softmax-row half h=c%2 (2048 rows each).
Host permutes x[b].T columns so each core's own row-half comes first (m axis
consistently permuted; softmax/final sum are permutation-invariant).  Host
sums the two per-core partials and adds bv (exact: per-core sum_m w[m]=0.5).

v6 structure (trace-driven):
- PSUM split into FOUR independent [128,1024] tiles L1/L2/R1/R2 and each
  half-tile exp split into two 1024-wide ACTIVATEs.  Tile-framework deps are
  PSUM-tile granular, so with 2048-col regions the chain
  exp -> w-matmuls -> cast -> S-staging serialized at 5.56us/tile.  With
  four tiles the w+cast only block L1 while L2/R2 staging and the other
  exps proceed -> the chain hides under ~4us of ACT queue.
- ACT pace: 4 x (1024+352)/1.2 + 2 READ_ACC = 5.16us/tile.  Row sums R come
  from accum_out on the p2 exps plus DVE tensor_reduce of the p1 E-quarters.
- w psum is CAST to a bf16 slot (DVE); accumulation into wacc on GPSIMD
  (SBUF-only engine!), rmat copies on GPSIMD; last-tile adds on DVE so the
  epilogue is not gated on the GPSIMD queue.
- V projection fused into the epilogue; V-casts on ACT (idle there), mults
  on DVE.  Output packed via 32x32 block transpose into 4 contiguous rows
  (a [128,1] strided out-DMA costs ~7us in ring/teardown time).
- x arrives as 8 contiguous [128,512] pieces (1KB DMA elements) issued from
  sync+gpsimd queues; weights/biases on the scalar queue.  PE HAM warm-up
  dummies + early exp-table load run during the DMA wait.

HW notes: bf16 everywhere PE-facing; matmul out <= 512 fp32 cols (one PSUM
bank); GPSIMD cannot touch PSUM; DVE reads at most one PSUM operand;
tensor_tensor_reduce faults on HW.
"""

import sys

sys.path.insert(0, "/opt/trn_rl_repo")

import ml_dtypes
import numpy as np

import concourse.bass as bass
import concourse.mybir as mybir
import concourse.tile as tile
from concourse import bacc

D = 128
N = 4096
B = 4
NCORES = 8
HALF = N // 2
RT = HALF // 128  # 16 row tiles per core

F32 = mybir.dt.float32
BF16 = mybir.dt.bfloat16
NPBF = ml_dtypes.bfloat16
AF = mybir.ActivationFunctionType
ALU = mybir.AluOpType


def build_nc():
    nc = bacc.Bacc()
    xt = nc.dram_tensor("xt", [8, D, 512], BF16, kind="ExternalInput")
    wqT = nc.dram_tensor("wqT", [D, D], BF16, kind="ExternalInput")
    wkT = nc.dram_tensor("wkT", [D, D], BF16, kind="ExternalInput")
    wvT = nc.dram_tensor("wvT", [D, D], BF16, kind="ExternalInput")
    bq = nc.dram_tensor("bq", [D, 1], F32, kind="ExternalInput")
    bk = nc.dram_tensor("bk", [D, 1], F32, kind="ExternalInput")
    out = nc.dram_tensor("out", [4, 32], F32, kind="ExternalOutput")

    with tile.TileContext(nc) as tc:
        with (
            tc.tile_pool(name="singles", bufs=1) as singles,
            tc.tile_pool(name="pp", bufs=1, space="PSUM") as pp,
            tc.tile_pool(name="epool", bufs=3) as epool,
        ):
            L1 = pp.tile([128, 1024], F32, tag="L1", name="L1")
            L2 = pp.tile([128, 1024], F32, tag="L2", name="L2")
            R1 = pp.tile([128, 1024], F32, tag="R1", name="R1")
            R2 = pp.tile([128, 1024], F32, tag="R2", name="R2")
            P1 = {0: L1, 1: R1}
            P2 = {0: L2, 1: R2}

            wq_sb = singles.tile([D, D], BF16, tag="wq", name="wq_sb")
            wk_sb = singles.tile([D, D], BF16, tag="wk", name="wk_sb")
            wv_sb = singles.tile([D, D], BF16, tag="wv", name="wv_sb")
            bqs = singles.tile([D, 1], F32, tag="bq", name="bqs")
            bks = singles.tile([D, 1], F32, tag="bk", name="bks")
            ones_sb = singles.tile([D, D], BF16, tag="ones", name="ones_sb")
            tl_out = singles.tile([D, 1], F32, tag="tl", name="tl_out")
            xt_sb = singles.tile([D, N], BF16, tag="xt", name="xt_sb")
            kt_sb = singles.tile([D, N], BF16, tag="kt", name="kt_sb")
            qt_sb = singles.tile([D, HALF], BF16, tag="qt", name="qt_sb")
            part = singles.tile([128, 4 * RT], F32, tag="part", name="part")
            Rcol = singles.tile([128, RT], F32, tag="R", name="Rcol")
            rr = singles.tile([128, RT], F32, tag="rr", name="rr")
            rmat = singles.tile([128, 2, 4, D], BF16, tag="rmat", name="rmat")
            wstore = singles.tile([128, 4, 512], BF16, tag="wst", name="wstore")
            wacc = singles.tile([128, 1024], BF16, tag="wacc", name="wacc")
            escr = singles.tile([128, 2048], F32, tag="escr", name="escr")
            vsb = singles.tile([128, 2048], BF16, tag="vsb", name="vsb")
            odump = singles.tile([128, 1024], F32, tag="odump", name="odump")
            opart = singles.tile([128, 4], F32, tag="opart", name="opart")
            o1 = singles.tile([128, 1], F32, tag="o1", name="o1")
            o128 = singles.tile([128, 32], F32, tag="o128", name="o128")
            o4x32 = singles.tile([128, 32], F32, tag="o4x32", name="o4x32")

            nc.vector.memset(ones_sb, 1.0)
            nc.vector.memset(o128, 0.0)
            nc.vector.memset(rmat, 0.0)
            nc.gpsimd.memset(wacc, 0.0)

            # ---- DMAs: contiguous 512-col pieces, sync+gpsimd queues ----
            nc.sync.dma_start(wk_sb, wkT[:, :])
            # lead pieces (cols 0-2047 gate K-left and Q) as 256-col halves
            # so they spread across more rings and arrive ~2us earlier
            for c in (0, 1, 2, 3):
                nc.sync.dma_start(
                    xt_sb[:, c * 512 : c * 512 + 256], xt[c, :, 0:256]
                )
                nc.gpsimd.dma_start(
                    xt_sb[:, c * 512 + 256 : (c + 1) * 512], xt[c, :, 256:512]
                )
            for c in (4, 6):
                nc.sync.dma_start(xt_sb[:, c * 512 : (c + 1) * 512], xt[c, :, :])
            for c in (5, 7):
                nc.gpsimd.dma_start(xt_sb[:, c * 512 : (c + 1) * 512], xt[c, :, :])
            nc.scalar.dma_start(wq_sb, wqT[:, :])
            nc.scalar.dma_start(bks, bk[:, :])
            nc.scalar.dma_start(bqs, bq[:, :])
            nc.scalar.dma_start(wv_sb, wvT[:, :])

            # early exp table load while DMAs land
            nc.scalar.activation(out=tl_out, in_=ones_sb[:, 0:1], func=AF.Exp)

            # PE HAM warm-up
            for _ in range(36):
                nc.tensor.matmul(R2[:, 0:128], ones_sb, ones_sb, start=True, stop=True)

            # ---- projections ----
            def proj(w_sb, src0, dst_lo, dst_hi):
                for g in range(2):
                    nc.tensor.matmul(
                        dst_lo[:, g * 512 : (g + 1) * 512],
                        w_sb,
                        xt_sb[:, src0 + g * 512 : src0 + (g + 1) * 512],
                        start=True,
                        stop=True,
                    )
                for g in range(2):
                    nc.tensor.matmul(
                        dst_hi[:, g * 512 : (g + 1) * 512],
                        w_sb,
                        xt_sb[:, src0 + 1024 + g * 512 : src0 + 1024 + (g + 1) * 512],
                        start=True,
                        stop=True,
                    )

            proj(wk_sb, 0, L1, L2)  # K left
            proj(wq_sb, 0, R1, R2)  # Q
            nc.vector.tensor_scalar_add(out=kt_sb[:, 0:1024], in0=L1, scalar1=bks)
            nc.vector.tensor_scalar_add(out=kt_sb[:, 1024:2048], in0=L2, scalar1=bks)
            nc.vector.tensor_scalar_add(out=qt_sb[:, 0:128], in0=R1[:, 0:128], scalar1=bqs)
            # bulk qt drains ride the idle ACT engine (free affine: in+bias)
            nc.scalar.activation(
                out=qt_sb[:, 128:1024], in_=R1[:, 128:1024], func=AF.Identity, bias=bqs
            )
            nc.scalar.activation(out=qt_sb[:, 1024:2048], in_=R2, func=AF.Identity, bias=bqs)
            proj(wk_sb, 2048, R1, R2)  # K right (after qt drained)
            nc.vector.tensor_scalar_add(out=kt_sb[:, 2048:3072], in0=R1, scalar1=bks)
            nc.vector.tensor_scalar_add(out=kt_sb[:, 3072:4096], in0=R2, scalar1=bks)

            E_tiles = {}

            def get_E(i):
                if i not in E_tiles:
                    E_tiles[i] = epool.tile([128, N], BF16, tag="E", name=f"E_{i}")
                return E_tiles[i]

            def emit_S(i, half, part_idx):
                """Stage S cols for row tile i into (L|R){1,2}."""
                reg = (P1 if part_idx == 1 else P2)[half]
                base = half * 2048 + (part_idx - 1) * 1024
                lhsT = qt_sb[:, i * 128 : (i + 1) * 128]
                for g in range(2):
                    nc.tensor.matmul(
                        reg[:, g * 512 : (g + 1) * 512],
                        lhsT,
                        kt_sb[:, base + g * 512 : base + (g + 1) * 512],
                        start=True,
                        stop=True,
                    )

            def emit_exp(i, half, part_idx):
                """1024-wide exp; the p2 parts carry the free accum_out."""
                reg = (P1 if part_idx == 1 else P2)[half]
                base = half * 2048 + (part_idx - 1) * 1024
                if part_idx == 2:
                    acc = part[:, 4 * i + half : 4 * i + half + 1]
                elif i == RT - 1:
                    # last tile: ACT is free afterwards; accum here skips the
                    # DVE treduces on the tail-critical rr(15) chain
                    acc = part[:, 4 * i + 2 + half : 4 * i + 3 + half]
                else:
                    acc = None
                nc.scalar.activation(
                    out=get_E(i)[:, base : base + 1024],
                    in_=reg,
                    func=AF.Exp,
                    accum_out=acc,
                )

            def emit_tr(i, half):
                """DVE row-sum of the p1 E-quarter (no ACT READ_ACC cost)."""
                base = half * 2048
                nc.vector.tensor_reduce(
                    out=part[:, 4 * i + 2 + half : 4 * i + 3 + half],
                    in_=get_E(i)[:, base : base + 1024],
                    axis=mybir.AxisListType.X,
                    op=ALU.add,
                )

            def emit_rr(i):
                nc.vector.tensor_reduce(
                    out=Rcol[:, i : i + 1],
                    in_=part[:, 4 * i : 4 * i + 4],
                    axis=mybir.AxisListType.X,
                    op=ALU.add,
                )
                nc.vector.reciprocal(out=rr[:, i : i + 1], in_=Rcol[:, i : i + 1])
                p = i % 2
                for j in range(4):
                    nc.gpsimd.tensor_copy(
                        out=rmat[:, p, j, 32 * j : 32 * j + 1], in_=rr[:, i : i + 1]
                    )

            def emit_w(i, half, accum_dve=False, reg=None):
                """Rank-1 contraction of E tile i, m-half `half`, into
                (L1|R1)[:, 0:512]; chunk j lands on partition 32j; CAST to a
                bf16 slot on DVE; accumulate on GPSIMD (DVE in the tail)."""
                if reg is None:
                    reg = P1[half]
                p = i % 2
                E = E_tiles[i]
                for j in range(4):
                    m0 = half * 2048 + j * 512
                    nc.tensor.matmul(
                        reg[:, 0:512],
                        rmat[:, p, j, :],
                        E[:, m0 : m0 + 512],
                        start=(j == 0),
                        stop=(j == 3),
                        skip_group_check=True,
                    )
                slot = (2 * i + half) % 4
                nc.vector.tensor_copy(out=wstore[:, slot, :], in_=reg[:, 0:512])
                eng = nc.vector if accum_dve else nc.gpsimd
                eng.tensor_add(
                    out=wacc[:, half * 512 : (half + 1) * 512],
                    in0=wacc[:, half * 512 : (half + 1) * 512],
                    in1=wstore[:, slot, :],
                )

            # ---- prologue: first tile ----
            emit_S(0, 0, 1)
            emit_S(0, 0, 2)
            emit_exp(0, 0, 1)
            emit_exp(0, 0, 2)
            emit_S(0, 1, 1)
            emit_S(0, 1, 2)
            emit_exp(0, 1, 1)
            emit_exp(0, 1, 2)

            # ---- main loop ----
            # L1-chain per tile: expL(i)p1 -> w(i-1,0) -> cast -> S(i+1,0)p1
            # hides under the ~4us of remaining ACT work before expL(i+1)p1.
            for i in range(RT):
                last = i + 1 >= RT
                if i >= 1:
                    emit_w(i - 1, 0, accum_dve=last)
                if not last:
                    emit_S(i + 1, 0, 1)
                    emit_S(i + 1, 0, 2)
                    emit_exp(i + 1, 0, 1)
                    emit_exp(i + 1, 0, 2)
                if i != RT - 1:
                    emit_tr(i, 0)
                if i >= 1:
                    emit_w(i - 1, 1, accum_dve=last)
                if not last:
                    emit_S(i + 1, 1, 1)
                    emit_S(i + 1, 1, 2)
                    emit_exp(i + 1, 1, 1)
                    emit_exp(i + 1, 1, 2)
                if i != RT - 1:
                    emit_tr(i, 1)
                emit_rr(i)

            # ---- epilogue: V fused; out[d] = (1/N) sum_m w[m] V0[m,d] ----
            # The last two w-groups re-home to R1/R2 so V staging for the
            # first chunk pair starts on L1/L2 right after the final exps,
            # overlapping the w tail.  ACT casts and DVE mults stream.
            def epi_stage_V(c):
                vreg = L1 if c % 2 == 0 else L2
                for g in range(2):
                    nc.tensor.matmul(
                        vreg[:, g * 512 : (g + 1) * 512],
                        wv_sb,
                        xt_sb[:, c * 1024 + g * 512 : c * 1024 + (g + 1) * 512],
                        start=True,
                        stop=True,
                    )

            def epi_stage_rep(c):
                hf, jb = c // 2, 2 * (c % 2)
                rreg = R1 if c % 2 == 0 else R2
                for jj in (jb, jb + 1):
                    nc.tensor.matmul(
                        rreg[:, (jj - jb) * 512 : (jj - jb + 1) * 512],
                        ones_sb[32 * jj : 32 * jj + 1, :],
                        wacc[32 * jj : 32 * jj + 1, hf * 512 : (hf + 1) * 512],
                        start=True,
                        stop=True,
                        tile_position=(32 * jj, 0),
                    )

            def epi_castV(c):
                half = c % 2
                nc.scalar.activation(
                    out=vsb[:, half * 1024 : (half + 1) * 1024],
                    in_=L1 if half == 0 else L2,
                    func=AF.Identity,
                )

            def epi_mult(c):
                half = c % 2
                nc.vector.tensor_mul(
                    out=escr[:, half * 1024 : (half + 1) * 1024],
                    in0=vsb[:, half * 1024 : (half + 1) * 1024],
                    in1=R1 if half == 0 else R2,
                )

            def epi_reduce(c):
                half = c % 2
                if half == 0:
                    nc.scalar.activation(
                        out=odump,
                        in_=escr[:, 0:1024],
                        func=AF.Identity,
                        accum_out=opart[:, c : c + 1],
                    )
                else:
                    nc.vector.tensor_reduce(
                        out=opart[:, c : c + 1],
                        in_=escr[:, 1024:2048],
                        axis=mybir.AxisListType.X,
                        op=ALU.add,
                    )

            epi_stage_V(0)
            epi_stage_V(1)
            epi_castV(0)
            epi_castV(1)
            emit_w(RT - 1, 0, accum_dve=True, reg=R1)
            emit_w(RT - 1, 1, accum_dve=True, reg=R2)
            epi_stage_rep(0)
            epi_stage_rep(1)
            epi_mult(0)
            epi_mult(1)
            epi_stage_V(2)
            epi_stage_V(3)
            epi_reduce(0)
            epi_reduce(1)
            epi_castV(2)
            epi_castV(3)
            epi_stage_rep(2)
            epi_stage_rep(3)
            epi_mult(2)
            epi_mult(3)
            epi_reduce(2)
            epi_reduce(3)
            nc.vector.tensor_reduce(out=o1, in_=opart, axis=mybir.AxisListType.X, op=ALU.add)
            nc.scalar.activation(out=o128[:, 0:1], in_=o1, func=AF.Identity, scale=1.0 / N)
            # pack to 4 contiguous 128B rows for a fast output DMA
            nc.vector.transpose(out=o4x32, in_=o128)
            for b in range(4):
                nc.sync.dma_start(out[b : b + 1, :], o4x32[32 * b : 32 * b + 1, :])

    nc.compile()
    return nc


_cache = {}


def get_nc():
    if "nc" not in _cache:
        _cache["nc"] = build_nc()
    return _cache["nc"]


def make_in_maps(x, Wq, bq, Wk, bk, Wv, bv):
    x = np.asarray(x, np.float32)
    wqT = np.ascontiguousarray(np.asarray(Wq, np.float32).T.astype(NPBF))
    wkT = np.ascontiguousarray(np.asarray(Wk, np.float32).T.astype(NPBF))
    wvT = np.ascontiguousarray(np.asarray(Wv, np.float32).T.astype(NPBF))
    bqc = np.ascontiguousarray(np.asarray(bq, np.float32).reshape(D, 1))
    bkc = np.ascontiguousarray(np.asarray(bk, np.float32).reshape(D, 1))
    in_maps = []
    for c in range(NCORES):
        b = c // 2
        h = c % 2
        xbT = x[b].T.astype(NPBF)
        xperm = np.concatenate(
            [xbT[:, h * HALF : (h + 1) * HALF], xbT[:, (1 - h) * HALF : (2 - h) * HALF]], axis=1
        )
        xp = np.ascontiguousarray(xperm.reshape(D, 8, 512).transpose(1, 0, 2))
        in_maps.append(
            {"xt": xp, "wqT": wqT, "wkT": wkT, "wvT": wvT, "bq": bqc, "bk": bkc}
        )
    return in_maps


def combine(results, bv):
    outs = [np.asarray(results[c]["out"]).reshape(D) for c in range(NCORES)]
    bvf = np.asarray(bv, np.float32).reshape(D)
    return np.stack([outs[2 * b] + outs[2 * b + 1] + bvf for b in range(B)]).astype(np.float32)


def run(inputs, trace=False, **kwargs):
    from concourse.bass_utils import run_bass_kernel_spmd

    nc = get_nc()
    in_maps = make_in_maps(**inputs)
    res = run_bass_kernel_spmd(nc, in_maps, core_ids=list(range(NCORES)), trace=trace, **kwargs)
    return combine(res.results, inputs["bv"]), res


def kernel(x, Wq, bq, Wk, bk, Wv, bv):
    out, _ = run(dict(x=x, Wq=Wq, bq=bq, Wk=Wk, bk=bk, Wv=Wv, bv=bv))
    return out
